# revision 9
# baseline (speedup 1.0000x reference)
"""Trainium2 Bass kernel for ExpertsChooseMaskedExpand MoE routing.

Math (reference):
    xd[b,e,c,i] = sum_t x[b,t,(e,i)] * dmask[b,t,e,c]            (dispatch)
    y[b,e,c,o]  = sum_i xd[b,e,c,i] * w[e,o,i] + bias[o]         (expert mm)
    out[b,t,o]  = sum_{e,c} y[b,e,c,o] * cmb[b,t,e,c]            (combine)

Restructured with combine applied before the weight matmul (155 GF
instead of 215), then MEAN-SPLIT so every device matmul can run in
fp8 DoubleRow (2 MACs/cell/cycle) without losing accuracy:

    cmb = q + cmb'          q[b,e,c] = mean_t cmb  (host, exact)
    z   = mu + nu           mu[b,(e,i)] = sum_c xd*q  (token-independent;
                            97% of z's variance), nu from cmb'
    out[b,t,o] = nu@wstack                            (device, all fp8)
               + s'[t]*bias[o] + mu@wstack[o] + s_q*bias[o]   (host, exact)

mu never touches the device: mu = sum_t (dm@q)[t] * x[t,:], all host
inputs. The device-side fp8 error only scales with nu (std ~120) while
the error tolerance scales with max|out| (dominated by the mu part,
~100x larger) — measured end-to-end rel err ~3e-3 vs the 2e-2 gate.

Device phases per core (8 cores = batch b x expert-pair h), all
matmuls fp8e4 DoubleRow (K=256 per matmul):
    0. 4 PE warmup matmuls start the HAM un-throttle early
    1. xd[c,j] = sum_t dm8[t,c]*x8[t,j]     contraction t: 4 tt-pairs
    2. nuT[j,t] = sum_c xd8[c,j]*cmbT8[c,t] contraction c: 2 ct-pairs
    3. outT[o,t] = sum_k w8[k,o]*nuT[k,t]   contraction k: 4 kt-pairs
PSUM evictions are scale-by-pow2 copies (split between the scalar and
vector engines), output stored bf16. DMA descriptors are batched (the
~630ns per-descriptor issue cost on the queue engines was the v3/v4
bottleneck at the head): one descriptor per expert for x/dm/cmbT',
weight tiles in pairs, output o-tiles as single [128,1024] descriptors.
The host sums the two K-half partials, rescales, and adds exact terms.
"""

import numpy as np
import ml_dtypes

B, T, E, C = 4, 1024, 4, 512
IN, OUT = 2048, 8192
P = 128
TT = T // P          # 8  t-tiles
CT = C // P          # 4  c-tiles per expert
JT = 4               # j-tiles per expert (i = 512)
EL = 2               # experts handled per core (expert-pair split)
KT = EL * JT         # 8 k-tiles for the fused matmul (K = 1024 per core)
KTP = KT // 2        # 4 DoubleRow k-tile pairs
OT = OUT // P        # 64 o-tiles of 128 (full output width per core)
OTG = OT // 2        # 32 weight-load groups (2 o-tiles per DMA)
TCH = 2              # t-chunks of 512

WS = 1024.0          # weight fp8 scale   (w*WS absmax ~117 < 240)
ZS = 0.0625          # nu fp8 scale       (nu*ZS absmax ~95 < 240)
XS = 16.0            # x fp8 scale        (x*XS absmax ~90 < 240)
DS = 128.0           # dmask fp8 scale    (dm*DS < 128)
CS = 256.0           # cmb' fp8 scale     (cmb'*CS absmax ~137 < 240)
ALPHA = 1.0 / (WS * ZS)
N_WARM = 4           # tiny HAM warmup ahead of the first real matmul

_CACHE = {}


def _build_nc():
    import concourse.mybir as mybir
    import concourse.tile as tile
    from concourse import bacc

    f32 = mybir.dt.float32
    f8 = mybir.dt.float8e4
    bf16 = mybir.dt.bfloat16
    DR = mybir.MatmulPerfMode.DoubleRow

    nc = bacc.Bacc("TRN2", target_bir_lowering=False, debug=False, num_devices=8)
    x_t = nc.dram_tensor("x", (T, EL * 512), f8, kind="ExternalInput")
    dm_t = nc.dram_tensor("dm", (T, EL, C), f8, kind="ExternalInput")
    cT_t = nc.dram_tensor("cmbT", (EL, C, T), f8, kind="ExternalInput")
    # w8[p, ot, m, i, oi] = fp8(WS * wstack[h*1024 + (2m+i)*128 + p, ot*128+oi])
    w8_t = nc.dram_tensor("w8", (P, OT, KTP, 2, P), f8, kind="ExternalInput")
    warm_t = nc.dram_tensor("warm", (P, 512), bf16, kind="ExternalInput")
    # out_pk[p, ot, tch, u] = bf16 of WS*ZS*(nu@w)[tch*512+u, ot*128+p]
    o_t = nc.dram_tensor("out", (P, OT, TCH, 512), bf16, kind="ExternalOutput")

    x_r = x_t.ap().rearrange("(tt p) f -> p tt f", p=P)        # [128, 8, 1024]
    dm_r = dm_t.ap().rearrange("(tt p) e c -> p tt e c", p=P)  # [128, 8, 2, 512]
    cT_r = cT_t.ap().rearrange("e (ct p) t -> p e ct t", p=P)  # [128, 2, 4, 1024]
    w8_r = w8_t.ap().rearrange(
        "p (g two) m i oi -> p g two m i oi", two=2
    )                                                          # [128,32,2,4,2,128]
    o_r = o_t.ap()                                             # [128, 64, 2, 512]

    with tile.TileContext(nc) as tc:
        with (
            tc.tile_pool(name="persist", bufs=1) as persist,
            tc.tile_pool(name="wp", bufs=6) as wp,
            tc.tile_pool(name="op", bufs=4) as op,
        ):
            zT = persist.tile([P, KT, T], f8)         # 8 KiB/partition
            warm_sb = persist.tile([P, 512], bf16)

            w_tiles = {}

            def load_w(g):
                t = wp.tile([P, 2, KTP, 2, P], f8, tag="w", name=f"w_{g}")
                nc.sync.dma_start(t, w8_r[:, g, :, :, :, :])
                w_tiles[g] = t

            # ---- Phase 0: tiny PE warmup (warm data goes via the idle ----
            # ---- gpsimd queue so it doesn't delay the x/dm descriptors) ----
            with tc.tile_pool(name="wm", bufs=1, space="PSUM") as wm:
                nc.gpsimd.dma_start(warm_sb, warm_t.ap())
                wps = wm.tile([P, 512], f32, tag="warm")
                for _ in range(N_WARM):
                    nc.tensor.matmul(
                        wps, warm_sb[:, :P], warm_sb[:, :], start=True, stop=True
                    )

            # ---- Phases 1+2: per-expert dispatch and combine (fp8 DR) ----
            with (
                tc.tile_pool(name="xdm", bufs=2) as xdm,
                tc.tile_pool(name="cp", bufs=2) as cp,
                tc.tile_pool(name="xdp", bufs=1) as xdp,
                tc.tile_pool(name="ps_a", bufs=4, space="PSUM") as ps_a,
                tc.tile_pool(name="ps_b", bufs=2, space="PSUM") as ps_b,
            ):
                # one DMA descriptor per tensor per expert (the ~630ns
                # per-descriptor issue cost dominates the head otherwise)
                xe, dme, ce = {}, {}, {}
                for e in range(EL):
                    xe[e] = xdm.tile([P, TT, 512], f8, tag="x", name=f"x_{e}")
                    dme[e] = xdm.tile([P, TT, 512], f8, tag="dm", name=f"dm_{e}")
                    nc.sync.dma_start(xe[e], x_r[:, :, e * 512 : (e + 1) * 512])
                    nc.sync.dma_start(dme[e], dm_r[:, :, e, :])
                    ce[e] = cp.tile([P, CT, T], f8, tag="c", name=f"c_{e}")
                    nc.gpsimd.dma_start(ce[e], cT_r[:, e, :, :])
                load_w(0)

                for e in range(EL):
                    # phase 1: xd[c, j] = sum_t dm[t, c] * x[t, j]
                    ps1 = [
                        ps_a.tile([P, 512], f32, tag="ps1", name=f"ps1_{e}_{ct}")
                        for ct in range(CT)
                    ]
                    for qt in range(4):        # tt-pair = DR pair
                        qs = slice(2 * qt, 2 * qt + 2)
                        for ct in range(CT):
                            nc.tensor.matmul(
                                ps1[ct],
                                dme[e][:, qs, ct * P : (ct + 1) * P],
                                xe[e][:, qs, :],
                                start=(qt == 0),
                                stop=(qt == 3),
                                perf_mode=DR,
                            )
                    # evict xd to fp8 at scale 1 (psum = XS*DS*xd)
                    xd_e = xdp.tile([P, CT, 512], f8, tag="xd")
                    for ct in range(CT):
                        nc.scalar.mul(xd_e[:, ct, :], ps1[ct], 1.0 / (XS * DS))

                    # phase 2: nuT[j, t] = sum_c xd[c, j] * cmbT'[c, t]
                    for th in range(2):
                        for jt in range(JT):
                            ps2 = ps_b.tile([P, 512], f32, tag="ps2")
                            for u in range(CT // 2):   # ct-pair = DR pair
                                nc.tensor.matmul(
                                    ps2,
                                    xd_e[:, 2 * u : 2 * u + 2,
                                         jt * P : (jt + 1) * P],
                                    ce[e][:, 2 * u : 2 * u + 2,
                                          th * 512 : (th + 1) * 512],
                                    start=(u == 0),
                                    stop=(u == CT // 2 - 1),
                                    perf_mode=DR,
                                )
                            # psum = CS*nu; evict to fp8 at scale ZS
                            nc.vector.tensor_scalar_mul(
                                zT[:, e * JT + jt, th * 512 : (th + 1) * 512],
                                ps2,
                                ZS / CS,
                            )
                    if e == 0:
                        load_w(1)

            # ---- Phase 3 (fp8 DoubleRow, transposed): ----
            # ---- outT[o,t] = sum_m sum_i w8[m,i].T @ nuT[2m+i] ----
            with tc.tile_pool(name="ps_c", bufs=8, space="PSUM") as ps_c:
                for ot in range(OT):
                    g = ot // 2
                    for pg in range(g, min(g + 5, OTG)):
                        if pg not in w_tiles:
                            load_w(pg)
                    psum = [
                        ps_c.tile([P, 512], f32, tag="ps3", name=f"ps3_{ot}_{i}")
                        for i in range(TCH)
                    ]
                    for m in range(KTP):
                        st = w_tiles[g][:, ot % 2, m, :, :]
                        for tch in range(TCH):
                            nc.tensor.matmul(
                                psum[tch],
                                st,
                                zT[:, 2 * m : 2 * m + 2,
                                   tch * 512 : (tch + 1) * 512],
                                start=(m == 0),
                                stop=(m == KTP - 1),
                                perf_mode=DR,
                            )
                    # pure psum->bf16 copies split across both engines,
                    # then ONE output descriptor per o-tile
                    o_sb = op.tile([P, TCH, 512], bf16, tag="o_sb")
                    nc.vector.tensor_copy(o_sb[:, 0, :], psum[0])
                    nc.scalar.copy(o_sb[:, 1, :], psum[1])
                    nc.gpsimd.dma_start(o_r[:, ot, :, :], o_sb)

    nc.compile()
    return nc


def _get_nc():
    if "nc" not in _CACHE:
        _CACHE["nc"] = _build_nc()
    return _CACHE["nc"]


def _prep_in_maps(x, combine_array, dispatch_mask, weight, bias):
    f8 = ml_dtypes.float8_e4m3
    x = np.ascontiguousarray(x, dtype=np.float32)
    dm = np.ascontiguousarray(dispatch_mask, dtype=np.float32)
    cmb = np.asarray(combine_array, dtype=np.float64)
    weight = np.asarray(weight, dtype=np.float64)
    bias = np.asarray(bias, dtype=np.float64)

    # mean-split of the combine weights over tokens (host, exact)
    q = cmb.mean(axis=1)                           # (B, E, C)
    cmbp = cmb - q[:, None]                        # zero token-mean
    sp = cmbp.sum(axis=(2, 3))                     # (B, T)  s' for the bias term
    s_q = q.sum(axis=(1, 2))                       # (B,)
    # exact corrections: mu = sum_t (dm@q)[t]*x[t]; C = mu@wstack + s_q*bias
    g = np.einsum('btec,bec->bte', dm.astype(np.float64), q)
    xr = x.astype(np.float64).reshape(B, T, E, IN // E)
    mu = np.einsum('bte,btei->bei', g, xr).reshape(B, IN)
    w_e = weight.reshape(E, OUT, IN // E)
    wstack = np.ascontiguousarray(w_e.transpose(0, 2, 1)).reshape(IN, OUT)
    corr = (mu @ wstack + s_q[:, None] * bias[None, :]).astype(np.float32)
    spb = sp.astype(np.float32)
    bias32 = bias.astype(np.float32)

    def q8(a, scale):
        return np.clip(a * scale, -240.0, 240.0).astype(f8)

    x8 = q8(x, XS)                                 # (B, T, IN)
    dm8 = q8(dm, DS)                               # (B, T, E, C)
    cmbT8 = q8(np.ascontiguousarray(cmbp.transpose(0, 2, 3, 1)), CS)  # (B,E,C,T)
    wq8 = q8(wstack, WS)
    w8 = []
    for h in range(2):
        wh = wq8[h * 1024 : (h + 1) * 1024, :].reshape(KT, P, OT, P)
        w8.append(
            np.ascontiguousarray(wh.transpose(1, 2, 0, 3)).reshape(P, OT, KTP, 2, P)
        )
    warm = np.zeros((P, 512), dtype=ml_dtypes.bfloat16)

    in_maps = []
    for k in range(8):
        b, h = k // 2, k % 2
        in_maps.append(
            {
                "x": np.ascontiguousarray(x8[b][:, h * 1024 : (h + 1) * 1024]),
                "dm": np.ascontiguousarray(dm8[b][:, 2 * h : 2 * h + 2, :]),
                "cmbT": np.ascontiguousarray(cmbT8[b][2 * h : 2 * h + 2]),
                "w8": w8[h],
                "warm": warm,
            }
        )
    return in_maps, (corr, spb, bias32)


def _enable_persistent_cache():
    try:
        import jax

        jax.config.update("jax_compilation_cache_dir", "/tmp/jax_neff_cache")
        jax.config.update("jax_persistent_cache_min_compile_time_secs", 1.0)
    except Exception:
        pass


def run_spmd(in_maps, trace=False, **kwargs):
    from concourse.bass_utils import run_bass_kernel_spmd

    _enable_persistent_cache()
    nc = _get_nc()
    return run_bass_kernel_spmd(
        nc, in_maps, core_ids=list(range(8)), trace=trace, **kwargs
    )


def kernel(x, combine_array, dispatch_mask, weight, bias, num_experts):
    assert int(num_experts) == E
    in_maps, (corr, spb, bias32) = _prep_in_maps(
        x, combine_array, dispatch_mask, weight, bias
    )
    try:
        res = run_spmd(in_maps)
    except Exception:
        # transient device errors (e.g. a wedged core from a prior run)
        # usually clear on retry with a freshly built program
        _CACHE.clear()
        res = run_spmd(in_maps)
    out = np.empty((B, T, OUT), dtype=np.float32)
    for b in range(B):
        pk = res.results[2 * b]["out"].astype(np.float32) + res.results[
            2 * b + 1
        ]["out"].astype(np.float32)
        # (P, OT, TCH, 512) -> (t, o); rescale, add exact host terms
        out[b] = (
            pk.transpose(2, 3, 1, 0).reshape(T, OUT) * ALPHA
            + spb[b][:, None] * bias32[None, :]
            + corr[b][None, :]
        )
    return out


# revision 16
# speedup vs baseline: 1.0213x; 1.0213x over previous
"""Trainium2 Bass kernel for ExpertsChooseMaskedExpand MoE routing.

Math (reference):
    xd[b,e,c,i] = sum_t x[b,t,(e,i)] * dmask[b,t,e,c]            (dispatch)
    y[b,e,c,o]  = sum_i xd[b,e,c,i] * w[e,o,i] + bias[o]         (expert mm)
    out[b,t,o]  = sum_{e,c} y[b,e,c,o] * cmb[b,t,e,c]            (combine)

Restructured with combine applied before the weight matmul (155 GF
instead of 215), then MEAN-SPLIT so every device matmul can run in
fp8 DoubleRow (2 MACs/cell/cycle) without losing accuracy:

    cmb = q + cmb'          q[b,e,c] = mean_t cmb  (host, exact)
    z   = mu + nu           mu[b,(e,i)] = sum_c xd*q  (token-independent;
                            97% of z's variance), nu from cmb'
    out[b,t,o] = nu@wstack                            (device, all fp8)
               + s'[t]*bias[o] + mu@wstack[o] + s_q*bias[o]   (host, exact)

mu never touches the device: mu = sum_t (dm@q)[t] * x[t,:], all host
inputs. The device-side fp8 error only scales with nu (std ~120) while
the error tolerance scales with max|out| (dominated by the mu part,
~100x larger) — measured end-to-end rel err ~3e-3 vs the 2e-2 gate.

Device phases per core (8 cores = batch b x expert-pair h), all
matmuls fp8e4 DoubleRow (K=256 per matmul):
    0. 4 PE warmup matmuls start the HAM un-throttle early
    1. xd[c,j] = sum_t dm8[t,c]*x8[t,j]     contraction t: 4 tt-pairs
    2. nuT[j,t] = sum_c xd8[c,j]*cmbT8[c,t] contraction c: 2 ct-pairs
    3. outT[o,t] = sum_k w8[k,o]*nuT[k,t]   contraction k: 4 kt-pairs
PSUM evictions are scale-by-pow2 copies (split between the scalar and
vector engines), output stored bf16. DMA descriptors are batched (the
~630ns per-descriptor issue cost on the queue engines was the v3/v4
bottleneck at the head): one descriptor per expert for x/dm/cmbT',
weight tiles in pairs, output o-tiles as single [128,1024] descriptors.
The host sums the two K-half partials, rescales, and adds exact terms.
"""

import numpy as np
import ml_dtypes

B, T, E, C = 4, 1024, 4, 512
IN, OUT = 2048, 8192
P = 128
TT = T // P          # 8  t-tiles
CT = C // P          # 4  c-tiles per expert
JT = 4               # j-tiles per expert (i = 512)
EL = 2               # experts handled per core (expert-pair split)
KT = EL * JT         # 8 k-tiles for the fused matmul (K = 1024 per core)
KTP = KT // 2        # 4 DoubleRow k-tile pairs
OT = OUT // P        # 64 o-tiles of 128 (full output width per core)
OTG = OT // 2        # 32 weight-load groups (2 o-tiles per DMA)
TCH = 2              # t-chunks of 512

WS = 1024.0          # weight fp8 scale   (w*WS absmax ~117 < 240)
ZS = 0.0625          # nu fp8 scale       (nu*ZS absmax ~95 < 240)
XS = 16.0            # x fp8 scale        (x*XS absmax ~90 < 240)
DS = 128.0           # dmask fp8 scale    (dm*DS < 128)
CS = 256.0           # cmb' fp8 scale     (cmb'*CS absmax ~137 < 240)
ALPHA = 1.0 / (WS * ZS)
N_WARM = 8           # HAM warmup matmuls (memset-fed, start right after preamble)

_CACHE = {}


def _build_nc():
    import concourse.mybir as mybir
    import concourse.tile as tile
    from concourse import bacc

    f32 = mybir.dt.float32
    f8 = mybir.dt.float8e4
    bf16 = mybir.dt.bfloat16
    DR = mybir.MatmulPerfMode.DoubleRow

    nc = bacc.Bacc("TRN2", target_bir_lowering=False, debug=False, num_devices=8)
    x_t = nc.dram_tensor("x", (T, EL * 512), f8, kind="ExternalInput")
    dm_t = nc.dram_tensor("dm", (T, EL, C), f8, kind="ExternalInput")
    cT_t = nc.dram_tensor("cmbT", (EL, C, T), f8, kind="ExternalInput")
    # w8[p, ot, m, i, oi] = fp8(WS * wstack[h*1024 + (2m+i)*128 + p, ot*128+oi])
    w8_t = nc.dram_tensor("w8", (P, OT, KTP, 2, P), f8, kind="ExternalInput")
    # out_pk[p, ot, tch, u] = bf16 of WS*ZS*(nu@w)[tch*512+u, ot*128+p]
    o_t = nc.dram_tensor("out", (P, OT, TCH, 512), bf16, kind="ExternalOutput")

    x_r = x_t.ap().rearrange("(tt p) f -> p tt f", p=P)        # [128, 8, 1024]
    dm_r = dm_t.ap().rearrange("(tt p) e c -> p tt e c", p=P)  # [128, 8, 2, 512]
    cT_r = cT_t.ap().rearrange("e (ct p) t -> p e ct t", p=P)  # [128, 2, 4, 1024]
    w8_r = w8_t.ap().rearrange(
        "p (g two) m i oi -> p g two m i oi", two=2
    )                                                          # [128,32,2,4,2,128]
    o_r = o_t.ap()                                             # [128, 64, 2, 512]

    with tile.TileContext(nc) as tc:
        with (
            tc.tile_pool(name="persist", bufs=1) as persist,
            tc.tile_pool(name="wp", bufs=6) as wp,
            tc.tile_pool(name="op", bufs=4) as op,
        ):
            zT = persist.tile([P, KT, T], f8)         # 8 KiB/partition
            warm_sb = persist.tile([P, 512], bf16)

            w_tiles = {}

            def load_w(g):
                t = wp.tile([P, 2, KTP, 2, P], f8, tag="w", name=f"w_{g}")
                nc.sync.dma_start(t, w8_r[:, g, :, :, :, :])
                w_tiles[g] = t

            # ---- Phase 0: PE warmup fed by a memset (no DMA dependency, ----
            # ---- so it starts right after the runtime preamble and the ----
            # ---- HAM un-throttles before the first real matmul) ----
            with tc.tile_pool(name="wm", bufs=1, space="PSUM") as wm:
                nc.vector.memset(warm_sb, 0)
                wps = wm.tile([P, 512], f32, tag="warm")
                for _ in range(N_WARM):
                    nc.tensor.matmul(
                        wps, warm_sb[:, :P], warm_sb[:, :], start=True, stop=True
                    )

            # ---- Phases 1+2: per-expert dispatch and combine (fp8 DR) ----
            with (
                tc.tile_pool(name="xdm", bufs=2) as xdm,
                tc.tile_pool(name="cp", bufs=2) as cp,
                tc.tile_pool(name="xdp", bufs=2) as xdp,
                tc.tile_pool(name="ps_a", bufs=4, space="PSUM") as ps_a,
                tc.tile_pool(name="ps_b", bufs=3, space="PSUM") as ps_b,
            ):
                # one DMA descriptor per tensor per expert (the ~630ns
                # per-descriptor issue cost dominates the head otherwise)
                xe, dme, ce = {}, {}, {}
                for e in range(EL):
                    xe[e] = xdm.tile([P, TT, 512], f8, tag="x", name=f"x_{e}")
                    dme[e] = xdm.tile([P, TT, 512], f8, tag="dm", name=f"dm_{e}")
                    if e == 0:
                        # split the first tt-pair out so the first real
                        # matmul's data lands as early as possible
                        nc.sync.dma_start(
                            xe[e][:, :2, :], x_r[:, :2, e * 512 : (e + 1) * 512]
                        )
                        nc.sync.dma_start(dme[e][:, :2, :], dm_r[:, :2, e, :])
                        nc.sync.dma_start(
                            xe[e][:, 2:, :], x_r[:, 2:, e * 512 : (e + 1) * 512]
                        )
                        nc.sync.dma_start(dme[e][:, 2:, :], dm_r[:, 2:, e, :])
                    else:
                        nc.sync.dma_start(xe[e], x_r[:, :, e * 512 : (e + 1) * 512])
                        nc.sync.dma_start(dme[e], dm_r[:, :, e, :])
                    ce[e] = cp.tile([P, CT, T], f8, tag="c", name=f"c_{e}")
                    nc.gpsimd.dma_start(ce[e], cT_r[:, e, :, :])
                load_w(0)

                for e in range(EL):
                    # phase 1: xd[c, j] = sum_t dm[t, c] * x[t, j]
                    # ct-outer so each psum's eviction overlaps the next
                    # ct's matmuls instead of stalling the PE at the end
                    xd_e = xdp.tile([P, CT, 512], f8, tag="xd", name=f"xd_{e}")
                    for ct in range(CT):
                        ps1 = ps_a.tile([P, 512], f32, tag="ps1",
                                        name=f"ps1_{e}_{ct}")
                        for qt in range(4):    # tt-pair = DR pair
                            qs = slice(2 * qt, 2 * qt + 2)
                            nc.tensor.matmul(
                                ps1,
                                dme[e][:, qs, ct * P : (ct + 1) * P],
                                xe[e][:, qs, :],
                                start=(qt == 0),
                                stop=(qt == 3),
                                perf_mode=DR,
                            )
                        # evict xd to fp8 at scale 1 (psum = XS*DS*xd)
                        nc.scalar.mul(xd_e[:, ct, :], ps1, 1.0 / (XS * DS))

                    # phase 2: nuT[j, t] = sum_c xd[c, j] * cmbT'[c, t]
                    # nu evictions alternate vector/scalar: one engine's
                    # ~690ns rate can't keep up with 432ns/psum production
                    for th in range(2):
                        for jt in range(JT):
                            ps2 = ps_b.tile([P, 512], f32, tag="ps2")
                            for u in range(CT // 2):   # ct-pair = DR pair
                                nc.tensor.matmul(
                                    ps2,
                                    xd_e[:, 2 * u : 2 * u + 2,
                                         jt * P : (jt + 1) * P],
                                    ce[e][:, 2 * u : 2 * u + 2,
                                          th * 512 : (th + 1) * 512],
                                    start=(u == 0),
                                    stop=(u == CT // 2 - 1),
                                    perf_mode=DR,
                                )
                            # psum = CS*nu; evict to fp8 at scale ZS
                            dst = zT[:, e * JT + jt, th * 512 : (th + 1) * 512]
                            if jt % 2 == 0:
                                nc.vector.tensor_scalar_mul(dst, ps2, ZS / CS)
                            else:
                                nc.scalar.mul(dst, ps2, ZS / CS)
                    if e == 0:
                        load_w(1)

            # ---- Phase 3 (fp8 DoubleRow, transposed): ----
            # ---- outT[o,t] = sum_m sum_i w8[m,i].T @ nuT[2m+i] ----
            with tc.tile_pool(name="ps_c", bufs=8, space="PSUM") as ps_c:
                for ot in range(OT):
                    g = ot // 2
                    for pg in range(g, min(g + 5, OTG)):
                        if pg not in w_tiles:
                            load_w(pg)
                    psum = [
                        ps_c.tile([P, 512], f32, tag="ps3", name=f"ps3_{ot}_{i}")
                        for i in range(TCH)
                    ]
                    for m in range(KTP):
                        st = w_tiles[g][:, ot % 2, m, :, :]
                        for tch in range(TCH):
                            nc.tensor.matmul(
                                psum[tch],
                                st,
                                zT[:, 2 * m : 2 * m + 2,
                                   tch * 512 : (tch + 1) * 512],
                                start=(m == 0),
                                stop=(m == KTP - 1),
                                perf_mode=DR,
                            )
                    # pure psum->bf16 copies split across both engines,
                    # then ONE output descriptor per o-tile
                    o_sb = op.tile([P, TCH, 512], bf16, tag="o_sb")
                    nc.vector.tensor_copy(o_sb[:, 0, :], psum[0])
                    nc.scalar.copy(o_sb[:, 1, :], psum[1])
                    nc.gpsimd.dma_start(o_r[:, ot, :, :], o_sb)

    nc.compile()
    return nc


def _get_nc():
    if "nc" not in _CACHE:
        _CACHE["nc"] = _build_nc()
    return _CACHE["nc"]


def _prep_in_maps(x, combine_array, dispatch_mask, weight, bias):
    f8 = ml_dtypes.float8_e4m3
    x = np.ascontiguousarray(x, dtype=np.float32)
    dm = np.ascontiguousarray(dispatch_mask, dtype=np.float32)
    cmb = np.asarray(combine_array, dtype=np.float64)
    weight = np.asarray(weight, dtype=np.float64)
    bias = np.asarray(bias, dtype=np.float64)

    # mean-split of the combine weights over tokens (host, exact)
    q = cmb.mean(axis=1)                           # (B, E, C)
    cmbp = cmb - q[:, None]                        # zero token-mean
    sp = cmbp.sum(axis=(2, 3))                     # (B, T)  s' for the bias term
    s_q = q.sum(axis=(1, 2))                       # (B,)
    # exact corrections: mu = sum_t (dm@q)[t]*x[t]; C = mu@wstack + s_q*bias
    g = np.einsum('btec,bec->bte', dm.astype(np.float64), q)
    xr = x.astype(np.float64).reshape(B, T, E, IN // E)
    mu = np.einsum('bte,btei->bei', g, xr).reshape(B, IN)
    w_e = weight.reshape(E, OUT, IN // E)
    wstack = np.ascontiguousarray(w_e.transpose(0, 2, 1)).reshape(IN, OUT)
    corr = (mu @ wstack + s_q[:, None] * bias[None, :]).astype(np.float32)
    spb = sp.astype(np.float32)
    bias32 = bias.astype(np.float32)

    def q8(a, scale):
        return np.clip(a * scale, -240.0, 240.0).astype(f8)

    x8 = q8(x, XS)                                 # (B, T, IN)
    dm8 = q8(dm, DS)                               # (B, T, E, C)
    cmbT8 = q8(np.ascontiguousarray(cmbp.transpose(0, 2, 3, 1)), CS)  # (B,E,C,T)
    wq8 = q8(wstack, WS)
    w8 = []
    for h in range(2):
        wh = wq8[h * 1024 : (h + 1) * 1024, :].reshape(KT, P, OT, P)
        w8.append(
            np.ascontiguousarray(wh.transpose(1, 2, 0, 3)).reshape(P, OT, KTP, 2, P)
        )
    in_maps = []
    for k in range(8):
        b, h = k // 2, k % 2
        in_maps.append(
            {
                "x": np.ascontiguousarray(x8[b][:, h * 1024 : (h + 1) * 1024]),
                "dm": np.ascontiguousarray(dm8[b][:, 2 * h : 2 * h + 2, :]),
                "cmbT": np.ascontiguousarray(cmbT8[b][2 * h : 2 * h + 2]),
                "w8": w8[h],
            }
        )
    return in_maps, (corr, spb, bias32)


def _enable_persistent_cache():
    try:
        import jax

        jax.config.update("jax_compilation_cache_dir", "/tmp/jax_neff_cache")
        jax.config.update("jax_persistent_cache_min_compile_time_secs", 1.0)
    except Exception:
        pass


def run_spmd(in_maps, trace=False, **kwargs):
    from concourse.bass_utils import run_bass_kernel_spmd

    _enable_persistent_cache()
    nc = _get_nc()
    return run_bass_kernel_spmd(
        nc, in_maps, core_ids=list(range(8)), trace=trace, **kwargs
    )


def kernel(x, combine_array, dispatch_mask, weight, bias, num_experts):
    assert int(num_experts) == E
    in_maps, (corr, spb, bias32) = _prep_in_maps(
        x, combine_array, dispatch_mask, weight, bias
    )
    try:
        res = run_spmd(in_maps)
    except Exception:
        # transient device errors (e.g. a wedged core from a prior run)
        # usually clear on retry with a freshly built program
        _CACHE.clear()
        res = run_spmd(in_maps)
    out = np.empty((B, T, OUT), dtype=np.float32)
    for b in range(B):
        pk = res.results[2 * b]["out"].astype(np.float32) + res.results[
            2 * b + 1
        ]["out"].astype(np.float32)
        # (P, OT, TCH, 512) -> (t, o); rescale, add exact host terms
        out[b] = (
            pk.transpose(2, 3, 1, 0).reshape(T, OUT) * ALPHA
            + spb[b][:, None] * bias32[None, :]
            + corr[b][None, :]
        )
    return out


# revision 21
# speedup vs baseline: 1.0635x; 1.0414x over previous
"""Trainium2 Bass kernel for ExpertsChooseMaskedExpand MoE routing.

Math (reference):
    xd[b,e,c,i] = sum_t x[b,t,(e,i)] * dmask[b,t,e,c]            (dispatch)
    y[b,e,c,o]  = sum_i xd[b,e,c,i] * w[e,o,i] + bias[o]         (expert mm)
    out[b,t,o]  = sum_{e,c} y[b,e,c,o] * cmb[b,t,e,c]            (combine)

Restructured with combine applied before the weight matmul (155 GF
instead of 215), then MEAN-SPLIT so every device matmul can run in
fp8 DoubleRow (2 MACs/cell/cycle) without losing accuracy:

    cmb = q + cmb'          q[b,e,c] = mean_t cmb  (host, exact)
    z   = mu + nu           mu[b,(e,i)] = sum_c xd*q  (token-independent;
                            97% of z's variance), nu from cmb'
    out[b,t,o] = nu@wstack                            (device, all fp8)
               + s'[t]*bias[o] + mu@wstack[o] + s_q*bias[o]   (host, exact)

mu never touches the device: mu = sum_t (dm@q)[t] * x[t,:], all host
inputs. The device-side fp8 error only scales with nu (std ~120) while
the error tolerance scales with max|out| (dominated by the mu part,
~100x larger) — measured end-to-end rel err ~3e-3 vs the 2e-2 gate.

Device phases per core (8 cores = batch b x expert-pair h), all
matmuls fp8e4 DoubleRow (K=256 per matmul):
    0. 4 PE warmup matmuls start the HAM un-throttle early
    1. xd[c,j] = sum_t dm8[t,c]*x8[t,j]     contraction t: 4 tt-pairs
    2. nuT[j,t] = sum_c xd8[c,j]*cmbT8[c,t] contraction c: 2 ct-pairs
    3. outT[o,t] = sum_k w8[k,o]*nuT[k,t]   contraction k: 4 kt-pairs
PSUM evictions are scale-by-pow2 copies (split between the scalar and
vector engines), output stored bf16. DMA descriptors are batched (the
~630ns per-descriptor issue cost on the queue engines was the v3/v4
bottleneck at the head): one descriptor per expert for x/dm/cmbT',
weight tiles in pairs, output o-tiles as single [128,1024] descriptors.
The host sums the two K-half partials, rescales, and adds exact terms.
"""

import numpy as np
import ml_dtypes

B, T, E, C = 4, 1024, 4, 512
IN, OUT = 2048, 8192
P = 128
TT = T // P          # 8  t-tiles
CT = C // P          # 4  c-tiles per expert
JT = 4               # j-tiles per expert (i = 512)
EL = 2               # experts handled per core (expert-pair split)
KT = EL * JT         # 8 k-tiles for the fused matmul (K = 1024 per core)
KTP = KT // 2        # 4 DoubleRow k-tile pairs
OT = OUT // P        # 64 o-tiles of 128 (full output width per core)
OTG = OT // 2        # 32 weight-load groups (2 o-tiles per DMA)
TCH = 2              # t-chunks of 512

WS = 1024.0          # weight fp8 scale   (w*WS absmax ~117 < 240)
ZS = 0.0625          # nu fp8 scale       (nu*ZS absmax ~95 < 240)
XS = 16.0            # x fp8 scale        (x*XS absmax ~90 < 240)
DS = 128.0           # dmask fp8 scale    (dm*DS < 128)
CS = 256.0           # cmb' fp8 scale     (cmb'*CS absmax ~137 < 240)
ALPHA = 1.0 / (WS * ZS)
N_WARM = 13          # memset-fed warmup matmuls bridging the ~5.5us from the
                     # runtime preamble to the first input-DMA completion

_CACHE = {}


def _build_nc():
    import concourse.mybir as mybir
    import concourse.tile as tile
    from concourse import bacc

    f32 = mybir.dt.float32
    f8 = mybir.dt.float8e4
    bf16 = mybir.dt.bfloat16
    DR = mybir.MatmulPerfMode.DoubleRow

    nc = bacc.Bacc("TRN2", target_bir_lowering=False, debug=False, num_devices=8)
    x_t = nc.dram_tensor("x", (T, EL * 512), f8, kind="ExternalInput")
    dm_t = nc.dram_tensor("dm", (T, EL, C), f8, kind="ExternalInput")
    cT_t = nc.dram_tensor("cmbT", (EL, C, T), f8, kind="ExternalInput")
    # w8[p, ot, m, i, oi] = fp8(WS * wstack[h*1024 + (2m+i)*128 + p, ot*128+oi])
    w8_t = nc.dram_tensor("w8", (P, OT, KTP, 2, P), f8, kind="ExternalInput")
    # out_pk[p, ot, tch, u] = bf16 of WS*ZS*(nu@w)[tch*512+u, ot*128+p]
    o_t = nc.dram_tensor("out", (P, OT, TCH, 512), bf16, kind="ExternalOutput")

    x_r = x_t.ap().rearrange("(tt p) f -> p tt f", p=P)        # [128, 8, 1024]
    dm_r = dm_t.ap().rearrange("(tt p) e c -> p tt e c", p=P)  # [128, 8, 2, 512]
    cT_r = cT_t.ap().rearrange("e (ct p) t -> p e ct t", p=P)  # [128, 2, 4, 1024]
    w8_r = w8_t.ap().rearrange(
        "p (g two) m i oi -> p g two m i oi", two=2
    )                                                          # [128,32,2,4,2,128]
    o_r = o_t.ap()                                             # [128, 64, 2, 512]

    with tile.TileContext(nc) as tc:
        with (
            tc.tile_pool(name="persist", bufs=1) as persist,
            tc.tile_pool(name="wp", bufs=6) as wp,
            tc.tile_pool(name="op", bufs=4) as op,
        ):
            zT = persist.tile([P, KT, T], f8)         # 8 KiB/partition
            warm_sb = persist.tile([P, 512], bf16)

            w_tiles = {}

            def load_w(g):
                t = wp.tile([P, 2, KTP, 2, P], f8, tag="w", name=f"w_{g}")
                nc.sync.dma_start(t, w8_r[:, g, :, :, :, :])
                w_tiles[g] = t

            # ---- Phase 0: PE warmup fed by a memset (no DMA dependency, ----
            # ---- so it starts right after the runtime preamble and the ----
            # ---- HAM un-throttles before the first real matmul) ----
            with tc.tile_pool(name="wm", bufs=1, space="PSUM") as wm:
                nc.vector.memset(warm_sb, 0)
                wps = wm.tile([P, 512], f32, tag="warm")
                for _ in range(N_WARM):
                    nc.tensor.matmul(
                        wps, warm_sb[:, :P], warm_sb[:, :], start=True, stop=True
                    )

            # ---- Phases 1+2: per-expert dispatch and combine (fp8 DR) ----
            with (
                tc.tile_pool(name="xdm", bufs=2) as xdm,
                tc.tile_pool(name="cp", bufs=2) as cp,
                tc.tile_pool(name="xdp", bufs=2) as xdp,
                tc.tile_pool(name="ps_a", bufs=4, space="PSUM") as ps_a,
                tc.tile_pool(name="ps_b", bufs=4, space="PSUM") as ps_b,
            ):
                # one DMA descriptor per tensor per expert (the ~630ns
                # per-descriptor issue cost dominates the head otherwise)
                xe, dme, ce = {}, {}, {}
                for e in range(EL):
                    xe[e] = xdm.tile([P, TT, 512], f8, tag="x", name=f"x_{e}")
                    dme[e] = xdm.tile([P, TT, 512], f8, tag="dm", name=f"dm_{e}")
                    if e == 0:
                        # three-way split so phase 1 streams: each tt-pair
                        # chunk feeds 4 matmuls while the next transfers
                        for sl in (slice(0, 2), slice(2, 4), slice(4, 8)):
                            nc.sync.dma_start(
                                xe[e][:, sl, :],
                                x_r[:, sl, e * 512 : (e + 1) * 512],
                            )
                            nc.sync.dma_start(dme[e][:, sl, :], dm_r[:, sl, e, :])
                    else:
                        nc.sync.dma_start(xe[e], x_r[:, :, e * 512 : (e + 1) * 512])
                        nc.sync.dma_start(dme[e], dm_r[:, :, e, :])
                    ce[e] = cp.tile([P, CT, T], f8, tag="c", name=f"c_{e}")
                    nc.gpsimd.dma_start(ce[e], cT_r[:, e, :, :])
                load_w(0)

                for e in range(EL):
                    # phase 1: xd[c, j] = sum_t dm[t, c] * x[t, j]
                    # e0: qt-outer so compute streams behind the arriving
                    # chunks; e1 (inputs long since loaded): ct-outer so
                    # each psum's eviction overlaps the next ct's matmuls
                    xd_e = xdp.tile([P, CT, 512], f8, tag="xd", name=f"xd_{e}")
                    ps1 = [
                        ps_a.tile([P, 512], f32, tag="ps1", name=f"ps1_{e}_{ct}")
                        for ct in range(CT)
                    ]
                    order = (
                        [(qt, ct) for qt in range(4) for ct in range(CT)]
                        if e == 0
                        else [(qt, ct) for ct in range(CT) for qt in range(4)]
                    )
                    for qt, ct in order:
                        qs = slice(2 * qt, 2 * qt + 2)
                        nc.tensor.matmul(
                            ps1[ct],
                            dme[e][:, qs, ct * P : (ct + 1) * P],
                            xe[e][:, qs, :],
                            start=(qt == 0),
                            stop=(qt == 3),
                            perf_mode=DR,
                        )
                        if qt == 3:
                            # evict xd to fp8 at scale 1 (psum = XS*DS*xd),
                            # engines alternating so pairs finish together
                            if ct % 2 == 0:
                                nc.scalar.mul(
                                    xd_e[:, ct, :], ps1[ct], 1.0 / (XS * DS)
                                )
                            else:
                                nc.vector.tensor_scalar_mul(
                                    xd_e[:, ct, :], ps1[ct], 1.0 / (XS * DS)
                                )

                    # phase 2: nuT[j, t] = sum_c xd[c, j] * cmbT'[c, t]
                    # jt-outer: each stationary xd pair serves both th
                    # chunks (1:2 LDW ratio like phase 3); the two psums
                    # evict concurrently on vector and scalar
                    for jt in range(JT):
                        ps2 = [
                            ps_b.tile([P, 512], f32, tag="ps2",
                                      name=f"ps2_{e}_{jt}_{th}")
                            for th in range(2)
                        ]
                        for u in range(CT // 2):   # ct-pair = DR pair
                            for th in range(2):
                                nc.tensor.matmul(
                                    ps2[th],
                                    xd_e[:, 2 * u : 2 * u + 2,
                                         jt * P : (jt + 1) * P],
                                    ce[e][:, 2 * u : 2 * u + 2,
                                          th * 512 : (th + 1) * 512],
                                    start=(u == 0),
                                    stop=(u == CT // 2 - 1),
                                    perf_mode=DR,
                                )
                        # psum = CS*nu; evict to fp8 at scale ZS
                        nc.vector.tensor_scalar_mul(
                            zT[:, e * JT + jt, 0:512], ps2[0], ZS / CS
                        )
                        nc.scalar.mul(
                            zT[:, e * JT + jt, 512:1024], ps2[1], ZS / CS
                        )
                    if e == 0:
                        load_w(1)

            # ---- Phase 3 (fp8 DoubleRow, transposed): ----
            # ---- outT[o,t] = sum_m sum_i w8[m,i].T @ nuT[2m+i] ----
            with tc.tile_pool(name="ps_c", bufs=8, space="PSUM") as ps_c:
                for ot in range(OT):
                    g = ot // 2
                    for pg in range(g, min(g + 5, OTG)):
                        if pg not in w_tiles:
                            load_w(pg)
                    psum = [
                        ps_c.tile([P, 512], f32, tag="ps3", name=f"ps3_{ot}_{i}")
                        for i in range(TCH)
                    ]
                    for m in range(KTP):
                        st = w_tiles[g][:, ot % 2, m, :, :]
                        for tch in range(TCH):
                            nc.tensor.matmul(
                                psum[tch],
                                st,
                                zT[:, 2 * m : 2 * m + 2,
                                   tch * 512 : (tch + 1) * 512],
                                start=(m == 0),
                                stop=(m == KTP - 1),
                                perf_mode=DR,
                            )
                    # pure psum->bf16 copies split across both engines,
                    # then ONE output descriptor per o-tile
                    o_sb = op.tile([P, TCH, 512], bf16, tag="o_sb")
                    nc.vector.tensor_copy(o_sb[:, 0, :], psum[0])
                    nc.scalar.copy(o_sb[:, 1, :], psum[1])
                    # alternate output queues (sync is mostly idle in p3)
                    eng = nc.gpsimd if ot % 2 == 0 else nc.sync
                    eng.dma_start(o_r[:, ot, :, :], o_sb)

    nc.compile()
    return nc


def _get_nc():
    if "nc" not in _CACHE:
        _CACHE["nc"] = _build_nc()
    return _CACHE["nc"]


def _prep_in_maps(x, combine_array, dispatch_mask, weight, bias):
    f8 = ml_dtypes.float8_e4m3
    x = np.ascontiguousarray(x, dtype=np.float32)
    dm = np.ascontiguousarray(dispatch_mask, dtype=np.float32)
    cmb = np.asarray(combine_array, dtype=np.float64)
    weight = np.asarray(weight, dtype=np.float64)
    bias = np.asarray(bias, dtype=np.float64)

    # mean-split of the combine weights over tokens (host, exact)
    q = cmb.mean(axis=1)                           # (B, E, C)
    cmbp = cmb - q[:, None]                        # zero token-mean
    sp = cmbp.sum(axis=(2, 3))                     # (B, T)  s' for the bias term
    s_q = q.sum(axis=(1, 2))                       # (B,)
    # exact corrections: mu = sum_t (dm@q)[t]*x[t]; C = mu@wstack + s_q*bias
    g = np.einsum('btec,bec->bte', dm.astype(np.float64), q)
    xr = x.astype(np.float64).reshape(B, T, E, IN // E)
    mu = np.einsum('bte,btei->bei', g, xr).reshape(B, IN)
    w_e = weight.reshape(E, OUT, IN // E)
    wstack = np.ascontiguousarray(w_e.transpose(0, 2, 1)).reshape(IN, OUT)
    corr = (mu @ wstack + s_q[:, None] * bias[None, :]).astype(np.float32)
    spb = sp.astype(np.float32)
    bias32 = bias.astype(np.float32)

    def q8(a, scale):
        return np.clip(a * scale, -240.0, 240.0).astype(f8)

    x8 = q8(x, XS)                                 # (B, T, IN)
    dm8 = q8(dm, DS)                               # (B, T, E, C)
    cmbT8 = q8(np.ascontiguousarray(cmbp.transpose(0, 2, 3, 1)), CS)  # (B,E,C,T)
    wq8 = q8(wstack, WS)
    w8 = []
    for h in range(2):
        wh = wq8[h * 1024 : (h + 1) * 1024, :].reshape(KT, P, OT, P)
        w8.append(
            np.ascontiguousarray(wh.transpose(1, 2, 0, 3)).reshape(P, OT, KTP, 2, P)
        )
    in_maps = []
    for k in range(8):
        b, h = k // 2, k % 2
        in_maps.append(
            {
                "x": np.ascontiguousarray(x8[b][:, h * 1024 : (h + 1) * 1024]),
                "dm": np.ascontiguousarray(dm8[b][:, 2 * h : 2 * h + 2, :]),
                "cmbT": np.ascontiguousarray(cmbT8[b][2 * h : 2 * h + 2]),
                "w8": w8[h],
            }
        )
    return in_maps, (corr, spb, bias32)


def _enable_persistent_cache():
    try:
        import jax

        jax.config.update("jax_compilation_cache_dir", "/tmp/jax_neff_cache")
        jax.config.update("jax_persistent_cache_min_compile_time_secs", 1.0)
    except Exception:
        pass


def run_spmd(in_maps, trace=False, **kwargs):
    from concourse.bass_utils import run_bass_kernel_spmd

    _enable_persistent_cache()
    nc = _get_nc()
    return run_bass_kernel_spmd(
        nc, in_maps, core_ids=list(range(8)), trace=trace, **kwargs
    )


def kernel(x, combine_array, dispatch_mask, weight, bias, num_experts):
    assert int(num_experts) == E
    in_maps, (corr, spb, bias32) = _prep_in_maps(
        x, combine_array, dispatch_mask, weight, bias
    )
    try:
        res = run_spmd(in_maps)
    except Exception:
        # transient device errors (e.g. a wedged core from a prior run)
        # usually clear on retry with a freshly built program
        _CACHE.clear()
        res = run_spmd(in_maps)
    out = np.empty((B, T, OUT), dtype=np.float32)
    for b in range(B):
        pk = res.results[2 * b]["out"].astype(np.float32) + res.results[
            2 * b + 1
        ]["out"].astype(np.float32)
        # (P, OT, TCH, 512) -> (t, o); rescale, add exact host terms
        out[b] = (
            pk.transpose(2, 3, 1, 0).reshape(T, OUT) * ALPHA
            + spb[b][:, None] * bias32[None, :]
            + corr[b][None, :]
        )
    return out


# revision 25
# speedup vs baseline: 1.0661x; 1.0024x over previous
"""Trainium2 Bass kernel for ExpertsChooseMaskedExpand MoE routing.

Math (reference):
    xd[b,e,c,i] = sum_t x[b,t,(e,i)] * dmask[b,t,e,c]            (dispatch)
    y[b,e,c,o]  = sum_i xd[b,e,c,i] * w[e,o,i] + bias[o]         (expert mm)
    out[b,t,o]  = sum_{e,c} y[b,e,c,o] * cmb[b,t,e,c]            (combine)

Restructured with combine applied before the weight matmul (155 GF
instead of 215), then MEAN-SPLIT so every device matmul can run in
fp8 DoubleRow (2 MACs/cell/cycle) without losing accuracy:

    cmb = q + cmb'          q[b,e,c] = mean_t cmb  (host, exact)
    z   = mu + nu           mu[b,(e,i)] = sum_c xd*q  (token-independent;
                            97% of z's variance), nu from cmb'
    out[b,t,o] = nu@wstack                            (device, all fp8)
               + s'[t]*bias[o] + mu@wstack[o] + s_q*bias[o]   (host, exact)

mu never touches the device: mu = sum_t (dm@q)[t] * x[t,:], all host
inputs. The device-side fp8 error only scales with nu (std ~120) while
the error tolerance scales with max|out| (dominated by the mu part,
~100x larger) — measured end-to-end rel err ~3e-3 vs the 2e-2 gate.

Device phases per core (8 cores = batch b x expert-pair h), all
matmuls fp8e4 DoubleRow (K=256 per matmul):
    0. 4 PE warmup matmuls start the HAM un-throttle early
    1. xd[c,j] = sum_t dm8[t,c]*x8[t,j]     contraction t: 4 tt-pairs
    2. nuT[j,t] = sum_c xd8[c,j]*cmbT8[c,t] contraction c: 2 ct-pairs
    3. outT[o,t] = sum_k w8[k,o]*nuT[k,t]   contraction k: 4 kt-pairs
PSUM evictions are scale-by-pow2 copies (split between the scalar and
vector engines), output stored bf16. DMA descriptors are batched (the
~630ns per-descriptor issue cost on the queue engines was the v3/v4
bottleneck at the head): one descriptor per expert for x/dm/cmbT',
weight tiles in pairs, output o-tiles as single [128,1024] descriptors.
The host sums the two K-half partials, rescales, and adds exact terms.
"""

import numpy as np
import ml_dtypes

B, T, E, C = 4, 1024, 4, 512
IN, OUT = 2048, 8192
P = 128
TT = T // P          # 8  t-tiles
CT = C // P          # 4  c-tiles per expert
JT = 4               # j-tiles per expert (i = 512)
EL = 2               # experts handled per core (expert-pair split)
KT = EL * JT         # 8 k-tiles for the fused matmul (K = 1024 per core)
KTP = KT // 2        # 4 DoubleRow k-tile pairs
OT = OUT // P        # 64 o-tiles of 128 (full output width per core)
OTG = OT // 2        # 32 weight-load groups (2 o-tiles per DMA)
TCH = 2              # t-chunks of 512

WS = 1024.0          # weight fp8 scale   (w*WS absmax ~117 < 240)
ZS = 0.0625          # nu fp8 scale       (nu*ZS absmax ~95 < 240)
XS = 16.0            # x fp8 scale        (x*XS absmax ~90 < 240)
DS = 128.0           # dmask fp8 scale    (dm*DS < 128)
CS = 256.0           # cmb' fp8 scale     (cmb'*CS absmax ~137 < 240)
ALPHA = 1.0 / (WS * ZS)
N_WARM = 13          # memset-fed warmup matmuls bridging the ~5.5us from the
                     # runtime preamble to the first input-DMA completion

_CACHE = {}


def _build_nc():
    import concourse.mybir as mybir
    import concourse.tile as tile
    from concourse import bacc

    f32 = mybir.dt.float32
    f8 = mybir.dt.float8e4
    bf16 = mybir.dt.bfloat16
    DR = mybir.MatmulPerfMode.DoubleRow

    nc = bacc.Bacc("TRN2", target_bir_lowering=False, debug=False, num_devices=8)
    # all inputs partition-major (host pre-transposed) so DMA lines are
    # 2-4 KiB contiguous per partition instead of 512 B
    x_t = nc.dram_tensor("x", (P, EL, TT, 512), f8, kind="ExternalInput")
    dm_t = nc.dram_tensor("dm", (P, EL, TT, 512), f8, kind="ExternalInput")
    cT_t = nc.dram_tensor("cmbT", (P, EL, CT, T), f8, kind="ExternalInput")
    # w8[p, ot, m, i, oi] = fp8(WS * wstack[h*1024 + (2m+i)*128 + p, ot*128+oi])
    w8_t = nc.dram_tensor("w8", (P, OT, KTP, 2, P), f8, kind="ExternalInput")
    # out_pk[p, ot, tch, u] = bf16 of WS*ZS*(nu@w)[tch*512+u, ot*128+p]
    o_t = nc.dram_tensor("out", (P, OT, TCH, 512), bf16, kind="ExternalOutput")

    x_r = x_t.ap()                                             # [128, 2, 8, 512]
    dm_r = dm_t.ap()                                           # [128, 2, 8, 512]
    cT_r = cT_t.ap()                                           # [128, 2, 4, 1024]
    w8_r = w8_t.ap().rearrange(
        "p (g two) m i oi -> p g two m i oi", two=2
    )                                                          # [128,32,2,4,2,128]
    o_r = o_t.ap()                                             # [128, 64, 2, 512]

    with tile.TileContext(nc) as tc:
        with (
            tc.tile_pool(name="persist", bufs=1) as persist,
            tc.tile_pool(name="wp", bufs=6) as wp,
            tc.tile_pool(name="op", bufs=4) as op,
        ):
            zT = persist.tile([P, KT, T], f8)         # 8 KiB/partition
            warm_sb = persist.tile([P, 512], bf16)

            w_tiles = {}

            def load_w(g):
                t = wp.tile([P, 2, KTP, 2, P], f8, tag="w", name=f"w_{g}")
                nc.sync.dma_start(t, w8_r[:, g, :, :, :, :])
                w_tiles[g] = t

            # ---- Phase 0: PE warmup fed by a memset (no DMA dependency, ----
            # ---- so it starts right after the runtime preamble and the ----
            # ---- HAM un-throttles before the first real matmul) ----
            with tc.tile_pool(name="wm", bufs=1, space="PSUM") as wm:
                nc.vector.memset(warm_sb, 0)
                wps = wm.tile([P, 512], f32, tag="warm")
                for _ in range(N_WARM):
                    nc.tensor.matmul(
                        wps, warm_sb[:, :P], warm_sb[:, :], start=True, stop=True
                    )

            # ---- Phases 1+2: per-expert dispatch and combine (fp8 DR) ----
            with (
                tc.tile_pool(name="xdm", bufs=2) as xdm,
                tc.tile_pool(name="cp", bufs=2) as cp,
                tc.tile_pool(name="xdp", bufs=2) as xdp,
                tc.tile_pool(name="ps_a", bufs=4, space="PSUM") as ps_a,
                tc.tile_pool(name="ps_b", bufs=4, space="PSUM") as ps_b,
            ):
                # one DMA descriptor per tensor per expert (the ~630ns
                # per-descriptor issue cost dominates the head otherwise)
                xe, dme, ce = {}, {}, {}
                for e in range(EL):
                    xe[e] = xdm.tile([P, TT, 512], f8, tag="x", name=f"x_{e}")
                    dme[e] = xdm.tile([P, TT, 512], f8, tag="dm", name=f"dm_{e}")
                    if e == 0:
                        # three-way split so phase 1 streams: each tt-pair
                        # chunk feeds 4 matmuls while the next transfers
                        for sl in (slice(0, 2), slice(2, 4), slice(4, 8)):
                            nc.sync.dma_start(xe[e][:, sl, :], x_r[:, e, sl, :])
                            nc.sync.dma_start(dme[e][:, sl, :], dm_r[:, e, sl, :])
                    else:
                        nc.sync.dma_start(xe[e], x_r[:, e, :, :])
                        nc.sync.dma_start(dme[e], dm_r[:, e, :, :])
                    ce[e] = cp.tile([P, CT, T], f8, tag="c", name=f"c_{e}")
                    nc.gpsimd.dma_start(ce[e], cT_r[:, e, :, :])
                load_w(0)

                for e in range(EL):
                    # phase 1: xd[c, j] = sum_t dm[t, c] * x[t, j]
                    # e0: qt-outer so compute streams behind the arriving
                    # chunks; e1 (inputs long since loaded): ct-outer so
                    # each psum's eviction overlaps the next ct's matmuls
                    xd_e = xdp.tile([P, CT, 512], f8, tag="xd", name=f"xd_{e}")
                    ps1 = [
                        ps_a.tile([P, 512], f32, tag="ps1", name=f"ps1_{e}_{ct}")
                        for ct in range(CT)
                    ]
                    order = (
                        [(qt, ct) for qt in range(4) for ct in range(CT)]
                        if e == 0
                        else [(qt, ct) for ct in range(CT) for qt in range(4)]
                    )
                    for qt, ct in order:
                        qs = slice(2 * qt, 2 * qt + 2)
                        nc.tensor.matmul(
                            ps1[ct],
                            dme[e][:, qs, ct * P : (ct + 1) * P],
                            xe[e][:, qs, :],
                            start=(qt == 0),
                            stop=(qt == 3),
                            perf_mode=DR,
                        )
                        if qt == 3:
                            # evict xd to fp8 at scale 1 (psum = XS*DS*xd),
                            # engines alternating so pairs finish together
                            if ct % 2 == 0:
                                nc.scalar.mul(
                                    xd_e[:, ct, :], ps1[ct], 1.0 / (XS * DS)
                                )
                            else:
                                nc.vector.tensor_scalar_mul(
                                    xd_e[:, ct, :], ps1[ct], 1.0 / (XS * DS)
                                )

                    # phase 2: nuT[j, t] = sum_c xd[c, j] * cmbT'[c, t]
                    # jt-outer: each stationary xd pair serves both th
                    # chunks (1:2 LDW ratio like phase 3); the two psums
                    # evict concurrently on vector and scalar
                    for jt in range(JT):
                        ps2 = [
                            ps_b.tile([P, 512], f32, tag="ps2",
                                      name=f"ps2_{e}_{jt}_{th}")
                            for th in range(2)
                        ]
                        for u in range(CT // 2):   # ct-pair = DR pair
                            for th in range(2):
                                nc.tensor.matmul(
                                    ps2[th],
                                    xd_e[:, 2 * u : 2 * u + 2,
                                         jt * P : (jt + 1) * P],
                                    ce[e][:, 2 * u : 2 * u + 2,
                                          th * 512 : (th + 1) * 512],
                                    start=(u == 0),
                                    stop=(u == CT // 2 - 1),
                                    perf_mode=DR,
                                )
                        # psum = CS*nu; evict to fp8 at scale ZS
                        nc.vector.tensor_scalar_mul(
                            zT[:, e * JT + jt, 0:512], ps2[0], ZS / CS
                        )
                        nc.scalar.mul(
                            zT[:, e * JT + jt, 512:1024], ps2[1], ZS / CS
                        )
                    if e == 0:
                        load_w(1)

            # ---- Phase 3 (fp8 DoubleRow, transposed): ----
            # ---- outT[o,t] = sum_m sum_i w8[m,i].T @ nuT[2m+i] ----
            with tc.tile_pool(name="ps_c", bufs=8, space="PSUM") as ps_c:
                for ot in range(OT):
                    g = ot // 2
                    for pg in range(g, min(g + 5, OTG)):
                        if pg not in w_tiles:
                            load_w(pg)
                    psum = [
                        ps_c.tile([P, 512], f32, tag="ps3", name=f"ps3_{ot}_{i}")
                        for i in range(TCH)
                    ]
                    for m in range(KTP):
                        st = w_tiles[g][:, ot % 2, m, :, :]
                        for tch in range(TCH):
                            nc.tensor.matmul(
                                psum[tch],
                                st,
                                zT[:, 2 * m : 2 * m + 2,
                                   tch * 512 : (tch + 1) * 512],
                                start=(m == 0),
                                stop=(m == KTP - 1),
                                perf_mode=DR,
                            )
                    # pure psum->bf16 copies split across both engines,
                    # then ONE output descriptor per o-tile
                    o_sb = op.tile([P, TCH, 512], bf16, tag="o_sb")
                    nc.vector.tensor_copy(o_sb[:, 0, :], psum[0])
                    nc.scalar.copy(o_sb[:, 1, :], psum[1])
                    # alternate output queues (sync is mostly idle in p3)
                    eng = nc.gpsimd if ot % 2 == 0 else nc.sync
                    eng.dma_start(o_r[:, ot, :, :], o_sb)

    nc.compile()
    return nc


def _get_nc():
    if "nc" not in _CACHE:
        _CACHE["nc"] = _build_nc()
    return _CACHE["nc"]


def _prep_in_maps(x, combine_array, dispatch_mask, weight, bias):
    f8 = ml_dtypes.float8_e4m3
    x = np.ascontiguousarray(x, dtype=np.float32)
    dm = np.ascontiguousarray(dispatch_mask, dtype=np.float32)
    cmb = np.asarray(combine_array, dtype=np.float64)
    weight = np.asarray(weight, dtype=np.float64)
    bias = np.asarray(bias, dtype=np.float64)

    # mean-split of the combine weights over tokens (host, exact)
    q = cmb.mean(axis=1)                           # (B, E, C)
    cmbp = cmb - q[:, None]                        # zero token-mean
    sp = cmbp.sum(axis=(2, 3))                     # (B, T)  s' for the bias term
    s_q = q.sum(axis=(1, 2))                       # (B,)
    # exact corrections: mu = sum_t (dm@q)[t]*x[t]; C = mu@wstack + s_q*bias
    g = np.einsum('btec,bec->bte', dm.astype(np.float64), q)
    xr = x.astype(np.float64).reshape(B, T, E, IN // E)
    mu = np.einsum('bte,btei->bei', g, xr).reshape(B, IN)
    w_e = weight.reshape(E, OUT, IN // E)
    wstack = np.ascontiguousarray(w_e.transpose(0, 2, 1)).reshape(IN, OUT)
    corr = (mu @ wstack + s_q[:, None] * bias[None, :]).astype(np.float32)
    spb = sp.astype(np.float32)
    bias32 = bias.astype(np.float32)

    def q8(a, scale):
        return np.clip(a * scale, -240.0, 240.0).astype(f8)

    x8 = q8(x, XS)                                 # (B, T, IN)
    dm8 = q8(dm, DS)                               # (B, T, E, C)
    cmbT8 = q8(np.ascontiguousarray(cmbp.transpose(0, 2, 3, 1)), CS)  # (B,E,C,T)
    wq8 = q8(wstack, WS)
    w8 = []
    for h in range(2):
        wh = wq8[h * 1024 : (h + 1) * 1024, :].reshape(KT, P, OT, P)
        w8.append(
            np.ascontiguousarray(wh.transpose(1, 2, 0, 3)).reshape(P, OT, KTP, 2, P)
        )
    in_maps = []
    for k in range(8):
        b, h = k // 2, k % 2
        # partition-major relayouts matching the dram tensor shapes
        xc = x8[b][:, h * 1024 : (h + 1) * 1024]               # (T, 1024)
        xc = np.ascontiguousarray(
            xc.reshape(TT, P, EL, 512).transpose(1, 2, 0, 3)
        )                                                      # (P, EL, TT, 512)
        dmc = dm8[b][:, 2 * h : 2 * h + 2, :]                  # (T, 2, 512)
        dmc = np.ascontiguousarray(
            dmc.reshape(TT, P, EL, 512).transpose(1, 2, 0, 3)
        )                                                      # (P, EL, TT, 512)
        cc = cmbT8[b][2 * h : 2 * h + 2]                       # (2, C, T)
        cc = np.ascontiguousarray(
            cc.reshape(EL, CT, P, T).transpose(2, 0, 1, 3)
        )                                                      # (P, EL, CT, T)
        in_maps.append({"x": xc, "dm": dmc, "cmbT": cc, "w8": w8[h]})
    return in_maps, (corr, spb, bias32)


def _enable_persistent_cache():
    try:
        import jax

        jax.config.update("jax_compilation_cache_dir", "/tmp/jax_neff_cache")
        jax.config.update("jax_persistent_cache_min_compile_time_secs", 1.0)
    except Exception:
        pass


def run_spmd(in_maps, trace=False, **kwargs):
    from concourse.bass_utils import run_bass_kernel_spmd

    _enable_persistent_cache()
    nc = _get_nc()
    return run_bass_kernel_spmd(
        nc, in_maps, core_ids=list(range(8)), trace=trace, **kwargs
    )


def kernel(x, combine_array, dispatch_mask, weight, bias, num_experts):
    assert int(num_experts) == E
    in_maps, (corr, spb, bias32) = _prep_in_maps(
        x, combine_array, dispatch_mask, weight, bias
    )
    try:
        res = run_spmd(in_maps)
    except Exception:
        # transient device errors (e.g. a wedged core from a prior run)
        # usually clear on retry with a freshly built program
        _CACHE.clear()
        res = run_spmd(in_maps)
    out = np.empty((B, T, OUT), dtype=np.float32)
    for b in range(B):
        pk = res.results[2 * b]["out"].astype(np.float32) + res.results[
            2 * b + 1
        ]["out"].astype(np.float32)
        # (P, OT, TCH, 512) -> (t, o); rescale, add exact host terms
        out[b] = (
            pk.transpose(2, 3, 1, 0).reshape(T, OUT) * ALPHA
            + spb[b][:, None] * bias32[None, :]
            + corr[b][None, :]
        )
    return out
